# revision 3
# baseline (speedup 1.0000x reference)
"""Trainium2 Bass kernel for nn_ChannelAttention (squeeze-excite).

Reference computation:
    s = mean(x, axis=(H, W))                    # [B, C]   global avg pool
    h = relu(bn1(s @ w1))                       # [B, Cr]  Cr = 16
    o = bn2(h @ w2)                             # [B, C]
    return o[:, None, None, :]                  # [B, 1, 1, C]

Strategy (data-parallel over batch, 8 cores x 8 samples). Per-core DMA
bandwidth is capped ~430 GB/s regardless of ring count (measured: single
HWDGE ring 403 GB/s, dual-ring interleaved 431 GB/s, coarse dual split and
SWDGE both WORSE), so the kernel streams x on BOTH HWDGE rings (sync +
scalar) in interleaved ~1.6MB column chunks — the empirically fastest
pattern — and spends the rest of the design on keeping every engine's
in-order queue free of tail work:
  - 4 sample-pair tiles [128, 12544] (49 rows/partition, sample boundary
    at partition 64), chunked c0/c2(a,b) on the sync ring and c1/c3(a,b)
    on the scalar ring. The params pack rides FIRST on the scalar ring
    (lands ~12us) so BN prep can run mid-stream.
  - Squeeze per pair: PE reduces 11 of the 24.5 512-col slices with an
    M=33 pair-indicator lhsT (PSUM rows {0,32}); DVE pre-reduces the
    other 13.5 with a chain of tensor_adds into a [128,512] partial that
    PE folds with one matmul. Work fits a ~15.3us pair cadence with slack
    on both engines.
  - No 512->256 fold and no separate gather source: after the ACT copy
    acc->SBUF, FOUR tiny K=33 one-hot matmuls per pair gather BOTH
    256-halves of each sample row directly into the transposed sT layout
    [128ch, 8samples] x2 (rows 1..31 of acc are exact zeros from the
    zero columns of the indicator, so no pre-zeroing is needed).
    Gathers are emitted incrementally (pair q's gathers after pair q+1's
    direct matmuls) so the kernel tail only carries the LAST pair's.
  - BN prep: ACT sqrt (+eps via const-pool immediate) -> DVE reciprocal
    -> gpsimd muls/subs building sc1/bi1 and the BN2-folded augmented
    operand w2bi, all emitted BEFORE any stream compute on those queues
    (runs ~12-17us, engines otherwise idle). Rsqrt on ACT is blocked by
    bass for accuracy. bi1/sc1 are re-copied through ACT so the final
    Relu's only cross-engine wait is the PE matmul (Activation encoding
    has room for one sync wait when bias is an AP).
  - Excite MLP on PE: g1[16,8] = w1.T @ sT (K=256 split in 2), BN1+ReLU
    +1/HW scale as one ScalarE activation, o[8,256] = h_ext.T @ w2bi
    with BN2 folded into w2bi rows (bias row at partition 32).
  - Pair 3 is the kernel tail: its late chunks are split smaller and
    biased to PE (which runs hot), closing the last PSUM ~2us after the
    final byte lands; the remaining serial chain is copy -> 4 gathers ->
    copies -> mm1 -> relu -> mm2 -> copy -> out DMA.
"""

import sys

if "/opt/trn_rl_repo" not in sys.path:
    sys.path.insert(0, "/opt/trn_rl_repo")

import numpy as np

B, H, W, C = 64, 56, 56, 256
CR = 16
NCORES = 8
BL = B // NCORES  # samples per core
HWP = H * W  # 3136 spatial positions
NPAIR = BL // 2  # 4 sample-pairs per core
PFD = 2 * HWP * C // 128  # 12544 free-dim elements per partition
PW = 1316  # packed parameter tensor width (see _pack_params)
EPS = 1e-3

_CACHE: dict = {}


def _build_nc():
    import concourse.bass as bass
    import concourse.tile as tile
    from concourse import bacc, mybir
    from contextlib import ExitStack

    f32 = mybir.dt.float32
    AF = mybir.ActivationFunctionType

    nc = bacc.Bacc("TRN2", target_bir_lowering=False, debug=False)

    x_d = nc.dram_tensor("x", [NPAIR, 128, PFD], f32, kind="ExternalInput")
    par_d = nc.dram_tensor("params", [128, PW], f32, kind="ExternalInput")
    out_d = nc.dram_tensor("out", [BL, C], f32, kind="ExternalOutput")

    # column chunk map (units of 512 cols; 24.5 units total per pair):
    # sync ring: c0 [0:3072) + c2 [6144:9728); scalar: c1 [3072:6144) +
    # c3 [9728:12544) (incl. the 256 tail) -> 13u vs 11.5u+params, balanced.
    C0, C1, C2, C3 = (0, 3072), (3072, 6144), (6144, 9728), (9728, PFD)

    with ExitStack() as ctx:
        tc = ctx.enter_context(tile.TileContext(nc))
        xp = ctx.enter_context(tc.tile_pool(name="xp", bufs=3))
        pp = ctx.enter_context(tc.tile_pool(name="pp", bufs=1))
        dvp = ctx.enter_context(tc.tile_pool(name="dvp", bufs=3))
        accp = ctx.enter_context(tc.tile_pool(name="accp", bufs=4, space="PSUM"))
        mlpp = ctx.enter_context(tc.tile_pool(name="mlpp", bufs=1, space="PSUM"))

        # ---- all stream DMA triggers first on both ring engines ----
        pt = pp.tile([128, PW], f32, tag="pt", name="pt")
        nc.scalar.dma_start(pt, par_d[:, :])

        xts = [
            xp.tile([128, PFD], f32, tag="xt", name=f"xt{q}", bufs=3)
            for q in range(NPAIR)
        ]
        for q in range(NPAIR - 1):
            xt = xts[q]
            nc.sync.dma_start(xt[:, C0[0] : C0[1]], x_d[q][:, C0[0] : C0[1]])
            nc.scalar.dma_start(xt[:, C1[0] : C1[1]], x_d[q][:, C1[0] : C1[1]])
            nc.sync.dma_start(xt[:, C2[0] : C2[1]], x_d[q][:, C2[0] : C2[1]])
            nc.scalar.dma_start(xt[:, C3[0] : C3[1]], x_d[q][:, C3[0] : C3[1]])
        # pair 3: late chunks split smaller so tail consumption tracks the
        # final bytes closely
        xt = xts[NPAIR - 1]
        q = NPAIR - 1
        nc.sync.dma_start(xt[:, C0[0] : C0[1]], x_d[q][:, C0[0] : C0[1]])
        nc.scalar.dma_start(xt[:, C1[0] : C1[1]], x_d[q][:, C1[0] : C1[1]])
        nc.sync.dma_start(xt[:, 6144:8704], x_d[q][:, 6144:8704])
        nc.sync.dma_start(xt[:, 8704:9728], x_d[q][:, 8704:9728])
        nc.scalar.dma_start(xt[:, 9728:12032], x_d[q][:, 9728:12032])
        nc.scalar.dma_start(xt[:, 12032:PFD], x_d[q][:, 12032:PFD])

        # ---- constants on gpsimd (idle engine, eff-1.0 memsets) ----
        # pair indicator, M=33: col 0 -> partitions 0..63 (first sample of
        # the pair) -> PSUM row 0; col 32 -> partitions 64..127 -> row 32
        # (compute-engine APs require 32-aligned partition bases)
        po = pp.tile([128, 33], f32, tag="po", name="po")
        nc.gpsimd.memset(po, 0.0)
        nc.gpsimd.memset(po[0:64, 0:1], 1.0)
        nc.gpsimd.memset(po[64:128, 32:33], 1.0)

        # gather rhs bank: oh33[32j, q, b] = 1 iff b == 2q + j (rows 1..31
        # multiply acc rows that are exact matmul zeros)
        oh33 = pp.tile([128, NPAIR, BL], f32, tag="oh33", name="oh33")
        nc.gpsimd.memset(oh33, 0.0)
        for qq in range(NPAIR):
            for jj in range(2):
                b = 2 * qq + jj
                nc.gpsimd.memset(oh33[32 * jj : 32 * jj + 1, qq, b : b + 1], 1.0)

        # BN2 is folded into the second MLP matmul: w2bi rows 0..15 hold
        # w2 * k2, row 32 the BN2 bias vector; rows 16..31 zero. h_ext gets
        # a ones row at 32 so o = h_ext.T @ w2bi computes bn2(h @ w2).
        w2bi = pp.tile([33, C], f32, tag="w2bi", name="w2bi")
        nc.gpsimd.memset(w2bi, 0.0)
        h_ext = pp.tile([33, BL], f32, tag="h_ext", name="h_ext")
        nc.gpsimd.memset(h_ext, 0.0)
        nc.gpsimd.memset(h_ext[32:33, :], 1.0)

        # ---- parameter views ----
        w1a = pt[:, 0:CR]
        w1b = pt[:, CR : 2 * CR]
        w2t = pt[0:CR, 32 : 32 + C]
        ga1 = pt[0:CR, 288:289]
        be1 = pt[0:CR, 289:290]
        mu1 = pt[0:CR, 290:291]
        va1 = pt[0:CR, 291:292]
        ga2 = pt[0:CR, 292 : 292 + C]
        va2 = pt[0:CR, 548 : 548 + C]
        ga2r = pt[32:33, 292 : 292 + C]
        va2r = pt[32:33, 548 : 548 + C]
        be2r = pt[32:33, 804 : 804 + C]
        mu2r = pt[32:33, 1060 : 1060 + C]

        # ---- BN prep, emitted BEFORE any stream compute on ACT/DVE/Pool
        # queues: params land ~12us (first on the scalar ring) and pair-0
        # compute only starts ~17us, so this runs in otherwise-idle time.
        # eps + var copies both come from gpsimd so each ACT sqrt has a
        # single sync wait (Activation encoding limit with an AP bias).
        eps = pp.tile([33, 1], f32, tag="eps", name="eps")
        nc.gpsimd.memset(eps, EPS)
        va1c = pp.tile([CR, 1], f32, tag="va1c", name="va1c")
        nc.gpsimd.tensor_copy(va1c, va1)
        va2c = pp.tile([CR, C], f32, tag="va2c", name="va2c")
        nc.gpsimd.tensor_copy(va2c, va2)
        va2rc = pp.tile([33, C], f32, tag="va2rc", name="va2rc")
        nc.gpsimd.tensor_copy(va2rc[32:33, :], va2r)
        srt1 = pp.tile([CR, 1], f32, tag="srt1", name="srt1")
        nc.scalar.activation(srt1, va1c, AF.Sqrt, bias=eps[0:CR])
        srt2 = pp.tile([CR, C], f32, tag="srt2", name="srt2")
        nc.scalar.activation(srt2, va2c, AF.Sqrt, bias=eps[0:CR])
        srt2r = pp.tile([33, C], f32, tag="srt2r", name="srt2r")
        nc.scalar.activation(srt2r[32:33, :], va2rc[32:33, :], AF.Sqrt, bias=eps[32:33])

        rst1 = pp.tile([CR, 1], f32, tag="rst1", name="rst1")
        nc.vector.reciprocal(rst1, srt1)
        rst2 = pp.tile([CR, C], f32, tag="rst2", name="rst2")
        nc.vector.reciprocal(rst2, srt2)
        rst2r = pp.tile([33, C], f32, tag="rst2r", name="rst2r")
        nc.vector.reciprocal(rst2r[32:33, :], srt2r[32:33, :])

        # scale1 = gamma1/sqrt(var1+eps)/HW, bias1 = beta1 - mean1*k1
        k1 = pp.tile([CR, 1], f32, tag="k1", name="k1")
        nc.gpsimd.tensor_mul(k1, ga1, rst1)
        sc1 = pp.tile([CR, 1], f32, tag="sc1", name="sc1")
        nc.gpsimd.tensor_scalar_mul(sc1, k1, 1.0 / HWP)
        tm1 = pp.tile([CR, 1], f32, tag="tm1", name="tm1")
        nc.gpsimd.tensor_mul(tm1, mu1, k1)
        bi1 = pp.tile([CR, 1], f32, tag="bi1", name="bi1")
        nc.gpsimd.tensor_sub(bi1, be1, tm1)
        k2 = pp.tile([CR, C], f32, tag="k2", name="k2")
        nc.gpsimd.tensor_mul(k2, ga2, rst2)
        nc.gpsimd.tensor_mul(w2bi[0:CR, :], w2t, k2)
        k2r = pp.tile([33, C], f32, tag="k2r", name="k2r")
        nc.gpsimd.tensor_mul(k2r[32:33, :], ga2r, rst2r[32:33, :])
        tm2r = pp.tile([33, C], f32, tag="tm2r", name="tm2r")
        nc.gpsimd.tensor_mul(tm2r[32:33, :], mu2r, k2r[32:33, :])
        nc.gpsimd.tensor_sub(w2bi[32:33, :], be2r, tm2r[32:33, :])

        # route bi1/sc1 through ACT so the Relu's only cross-engine wait
        # is the PE matmul result
        bi1c = pp.tile([CR, 1], f32, tag="bi1c", name="bi1c")
        nc.scalar.copy(bi1c, bi1)
        sc1c = pp.tile([CR, 1], f32, tag="sc1c", name="sc1c")
        nc.scalar.copy(sc1c, sc1)

        # ---- stage 1: squeeze ----
        # acc_sb[32j, q*512:(q+1)*512]: raw [1,512] channel sums (row-pair
        # interleaved: cols 0:256 even rows + tail, 256:512 odd rows)
        acc_sb = pp.tile([128, NPAIR * 512], f32, tag="acc_sb", name="acc_sb")
        sT0 = mlpp.tile([128, BL], f32, tag="sT0", name="sT0")
        sT1 = mlpp.tile([128, BL], f32, tag="sT1", name="sT1")

        # per-pair engine split (units of 512 cols):
        #   PE direct: c0 slices 0..5 and c2 slices 12..16
        #   DVE chain: c1 slices 6..11, c2 slices 17..18, c3 slices
        #              19..23 + the 256 tail  (13 adds)
        PE_SL = list(range(0, 6)) + list(range(12, 17))
        DV_SL = list(range(6, 12)) + list(range(17, 24))

        accs = []
        gathers = []  # deferred per-pair gather emitters

        def emit_gathers(qq):
            base = qq * 512
            for hh, sT in enumerate((sT0, sT1)):
                for half in (0, 256):
                    nc.tensor.matmul(
                        sT[:, 0:BL],
                        acc_sb[0:33, base + half + hh * 128 : base + half + hh * 128 + 128],
                        oh33[0:33, qq, :],
                        start=(qq == 0 and half == 0),
                        stop=(qq == NPAIR - 1 and half == 256),
                    )

        for q in range(NPAIR):
            xt = xts[q]
            acc = accp.tile([128, 512], f32, tag="acc", name=f"acc{q}")
            accs.append(acc)
            last = q == NPAIR - 1

            # PE directs, in chunk-arrival order
            for i, k in enumerate(PE_SL):
                nc.tensor.matmul(
                    acc[0:33, :],
                    po,
                    xt[:, k * 512 : (k + 1) * 512],
                    start=(i == 0),
                    stop=False,
                )

            # DVE chain into a rotating partial
            dve_acc = dvp.tile([128, 512], f32, tag="dve_acc", name=f"dve{q}", bufs=3)
            nc.vector.tensor_add(
                dve_acc,
                xt[:, DV_SL[0] * 512 : (DV_SL[0] + 1) * 512],
                xt[:, DV_SL[1] * 512 : (DV_SL[1] + 1) * 512],
            )
            for k in DV_SL[2:]:
                nc.vector.tensor_add(
                    dve_acc, dve_acc, xt[:, k * 512 : (k + 1) * 512]
                )
            # 256 tail (row 48) folds into the even-row half
            nc.vector.tensor_add(
                dve_acc[:, 0:256], dve_acc[:, 0:256], xt[:, 24 * 512 : PFD]
            )

            # previous pair's gathers keep PE busy while this DVE chain runs
            if q >= 1:
                gathers.pop(0)()

            nc.tensor.matmul(acc[0:33, :], po, dve_acc, start=False, stop=True)
            nc.scalar.copy(acc_sb[0:33, q * 512 : (q + 1) * 512], acc[0:33, :])
            gathers.append(lambda qq=q: emit_gathers(qq))

        gathers.pop(0)()

        # ---- stage 2: excite MLP ----
        sT0s = pp.tile([128, BL], f32, tag="sT0s", name="sT0s")
        nc.scalar.copy(sT0s, sT0)
        sT1s = pp.tile([128, BL], f32, tag="sT1s", name="sT1s")
        nc.vector.tensor_copy(sT1s, sT1)

        g1p = mlpp.tile([CR, BL], f32, tag="g1p", name="g1p")
        nc.tensor.matmul(g1p, w1a, sT0s, start=True, stop=False)
        nc.tensor.matmul(g1p, w1b, sT1s, start=False, stop=True)

        # h = relu(g1 * scale1 + bias1): BN1 + mean scale + relu in one op
        nc.scalar.activation(h_ext[0:CR, :], g1p, AF.Relu, bias=bi1c, scale=sc1c)

        o_p = mlpp.tile([BL, C], f32, tag="o_p", name="o_p")
        nc.tensor.matmul(o_p, h_ext[0:33, 0:BL], w2bi[0:33, :], start=True, stop=True)

        ofin = pp.tile([BL, C], f32, tag="ofin", name="ofin")
        nc.scalar.copy(ofin, o_p)
        nc.sync.dma_start(out_d[:, :], ofin)

    nc.compile()
    return nc


def _get_nc():
    if "nc" not in _CACHE:
        _CACHE["nc"] = _build_nc()
    return _CACHE["nc"]


def _pack_params(inputs):
    def g(k):
        return np.asarray(inputs[k], dtype=np.float32)

    p = np.zeros((128, PW), np.float32)
    w1 = g("w1")
    p[:, 0:CR] = w1[0:128]
    p[:, CR : 2 * CR] = w1[128:256]
    p[0:CR, 32 : 32 + C] = g("w2")
    p[0:CR, 288] = g("gamma1")
    p[0:CR, 289] = g("beta1")
    p[0:CR, 290] = g("mean1")
    p[0:CR, 291] = g("var1")
    p[0:CR, 292 : 292 + C] = g("gamma2")[None, :]
    p[0:CR, 548 : 548 + C] = g("var2")[None, :]
    p[32, 292 : 292 + C] = g("gamma2")
    p[32, 548 : 548 + C] = g("var2")
    p[32, 804 : 804 + C] = g("beta2")
    p[32, 1060 : 1060 + C] = g("mean2")
    return p


def _in_maps(inputs):
    x = np.ascontiguousarray(np.asarray(inputs["x"], dtype=np.float32))
    params = _pack_params(inputs)
    maps = []
    for c in range(NCORES):
        shard = np.ascontiguousarray(x[c * BL : (c + 1) * BL]).reshape(NPAIR, 128, PFD)
        maps.append({"x": shard, "params": params})
    return maps


def _run(inputs, trace=False):
    from concourse.bass_utils import run_bass_kernel_spmd

    nc = _get_nc()
    res = run_bass_kernel_spmd(
        nc, _in_maps(inputs), core_ids=list(range(NCORES)), trace=trace
    )
    out = np.concatenate([res.results[c]["out"] for c in range(NCORES)], axis=0)
    return out.reshape(B, 1, 1, C).astype(np.float32), res


def kernel(**inputs) -> np.ndarray:
    out, _ = _run(inputs, trace=False)
    return out


def kernel_traced(**inputs):
    """Returns (out, BassKernelResults) with NTFF profiling enabled."""
    return _run(inputs, trace=True)


def bench(inputs, iters=30, warmup=5):
    """Time the per-step NEFF execution with device-resident inputs.

    Returns (out_full, per_call_seconds_list). Inputs are device_put once;
    each timed call only dispatches the compiled executable, so steady-state
    per-call wall time ~= max-core NEFF exec + dispatch overhead.
    """
    import time
    import jax
    import jax.numpy as jnp
    from jax.sharding import Mesh, PartitionSpec, NamedSharding
    from jax.experimental.shard_map import shard_map
    from concourse import bass2jax, mybir

    bass2jax.install_neuronx_cc_hook()
    nc = _get_nc()

    partition_name = nc.partition_id_tensor.name if nc.partition_id_tensor else None
    in_names, out_names, out_avals = [], [], []
    for alloc in nc.m.functions[0].allocations:
        if not isinstance(alloc, mybir.MemoryLocationSet):
            continue
        name = alloc.memorylocations[0].name
        if alloc.kind == "ExternalInput":
            if name != partition_name:
                in_names.append(name)
        elif alloc.kind == "ExternalOutput":
            out_names.append(name)
            out_avals.append(
                jax.core.ShapedArray(tuple(alloc.tensor_shape), mybir.dt.np(alloc.dtype))
            )
    all_in_names = in_names + out_names
    if partition_name is not None:
        all_in_names = all_in_names + [partition_name]

    def _body(*operands):
        operands = list(operands)
        if partition_name is not None:
            operands.append(bass2jax.partition_id_tensor())
        outs = bass2jax._bass_exec_p.bind(
            *operands,
            out_avals=tuple(out_avals),
            in_names=tuple(all_in_names),
            out_names=tuple(out_names),
            lowering_input_output_aliases=(),
            sim_require_finite=True,
            sim_require_nnan=True,
            nc=nc,
        )
        return tuple(outs)

    devices = jax.devices()[:NCORES]
    mesh = Mesh(np.asarray(devices), ("core",))
    spec = PartitionSpec("core")
    maps = _in_maps(inputs)
    concat = [
        np.concatenate([maps[c][n] for c in range(NCORES)], axis=0) for n in in_names
    ]
    concat += [
        np.zeros((NCORES * a.shape[0], *a.shape[1:]), a.dtype) for a in out_avals
    ]
    sharding = NamedSharding(mesh, spec)
    dev_in = [jax.device_put(a, sharding) for a in concat]

    fn = jax.jit(
        shard_map(
            _body,
            mesh=mesh,
            in_specs=(spec,) * len(concat),
            out_specs=(spec,) * len(out_names),
            check_rep=False,
        )
    )

    for _ in range(warmup):
        outs = fn(*dev_in)
    jax.block_until_ready(outs)

    times = []
    for _ in range(iters):
        t0 = time.perf_counter()
        outs = fn(*dev_in)
        jax.block_until_ready(outs)
        times.append(time.perf_counter() - t0)

    oidx = out_names.index("out")
    o = np.asarray(outs[oidx]).reshape(NCORES, BL, C).reshape(B, C)
    return o.reshape(B, 1, 1, C).astype(np.float32), times
